# revision 5
# baseline (speedup 1.0000x reference)
"""Symmetric-halved Euclidean distance matrix on 8 Trainium2 NeuronCores.

v2: fp8e4m3 DoubleRow gram matmuls (2x contraction per pass), norms folded in
on-device (row term via K=1 fp16 matmuls or DVE adds, column term via the
activation bias), sqrt on the scalar engine, fp16 output, host-precomputed
norms. 136 unique strip pairs (16 column strips of 512) are distributed 17 per
core: core c owns stars centered at strips 2c (offsets 0-7) and 2c+1 (offsets
0-7), plus the distance-8 pair {c, c+8}. The host mirrors each [512, 512]
block to its transposed position.

Per-core input is a local window of 11 strips (slots 0-8 = strips
2c..2c+8 mod 16, slot 9 = strip c+8, slot 10 = strip c), so the program is
SPMD-uniform with local slot addressing.
"""
import sys

sys.path.insert(0, "/opt/trn_rl_repo")

import numpy as np
import ml_dtypes

N, D, NCORES = 8192, 512, 8
P = 128
KO = D // P          # 4 contraction blocks of 128
NSTRIP = 16
SW = N // NSTRIP     # 512 strip width
NSLOT = 11           # local strips per core
NBLK = 17            # output blocks per core

# pair-groups: (center slot offset in x8 cols, [partner slots], bias base col,
#               j-term engine 'pe' | 'dve'). DVE-epilogue groups run mid-kernel
# so their add+sqrt chains overlap PE work; PE-j groups close the schedule to
# keep the pipeline drain short.
PGS = [
    (0, [0, 1], 0, "pe"),
    (0, [2, 3], 0, "pe"),
    (SW, [1, 2], 4, "dve"),
    (SW, [3, 4], 4, "dve"),
    (0, [4, 5], 0, "dve"),
    (0, [6, 7], 0, "dve"),
    (SW, [5, 6], 4, "pe"),
    (SW, [7, 8], 4, "pe"),
    (10 * SW, [9], 8, "pe"),
]

TRACE = False
LAST_EXEC_NS = None
LAST_RESULTS = None

_nc_cache = None


def _build():
    global _nc_cache
    if _nc_cache is not None:
        return _nc_cache

    import concourse.tile as tile
    from concourse import bacc, mybir

    f32 = mybir.dt.float32
    fp8 = mybir.dt.float8e4
    fp16 = mybir.dt.float16
    AF = mybir.ActivationFunctionType
    Alu = mybir.AluOpType
    DR = mybir.MatmulPerfMode.DoubleRow

    nc = bacc.Bacc("TRN2", target_bir_lowering=False)
    x8_d = nc.declare_dram_parameter("x8", [D, NSLOT * SW], fp8, isOutput=False)
    sqr_d = nc.declare_dram_parameter("sqr", [1, NSLOT * SW], fp16, isOutput=False)
    sqb_d = nc.declare_dram_parameter("sqb", [P, NSLOT * SW], fp16, isOutput=False)
    bias_d = nc.declare_dram_parameter("bias", [P, 12], f32, isOutput=False)
    out_d = nc.declare_dram_parameter("out", [NBLK * P, KO * SW], fp16, isOutput=True)

    with tile.TileContext(nc) as tc:
        with (
            tc.tile_pool(name="res", bufs=1) as res,
            tc.tile_pool(name="stg", bufs=3) as stg,
            tc.tile_pool(name="mmps", bufs=4, space="PSUM") as mmps,
        ):
            x8 = res.tile([P, KO, NSLOT * SW], fp8, tag="x8")
            sqr = res.tile([1, NSLOT, SW], fp16, tag="sqr")
            sqb = res.tile([P, NSLOT, SW], fp16, tag="sqb")
            bias = res.tile([P, 12], f32, tag="bias")
            ones = res.tile([1, P], fp16, tag="ones")

            nc.vector.memset(ones, 1.0)
            nc.sync.dma_start(sqr, sqr_d[:].rearrange("o (s j) -> o s j", j=SW))
            nc.sync.dma_start(bias, bias_d[:])
            x8_ap = x8_d[:]

            def load_x8(c0, c1):
                nc.sync.dma_start(
                    x8[:, :, c0:c1],
                    x8_ap[:, c0:c1].rearrange("(ko p) j -> p ko j", p=P),
                )

            # fine-grained so the first pair-groups start early; sqb goes on
            # the vector queue in parallel (first DVE add is ~10us in)
            nc.scalar.dma_start(sqb, sqb_d[:].rearrange("p (s j) -> p s j", j=SW))
            load_x8(0, 2 * SW)                       # slots 0-1
            load_x8(2 * SW, 4 * SW)                  # slots 2-3
            load_x8(4 * SW, 6 * SW)                  # slots 4-5
            load_x8(6 * SW, 9 * SW)                  # slots 6-8
            load_x8(9 * SW, 11 * SW)                 # slots 9-10

            out_v = out_d[:].rearrange("(b p) w -> b p w", p=P)

            for pgi, (coff, partners, bcol, jeng) in enumerate(PGS):
                npart = len(partners)
                stage = stg.tile([P, 2, KO, SW], fp16, tag="stage",
                                 name=f"stage{pgi}")
                for q in range(KO):
                    ps = mmps.tile([P, 2, SW], f32, tag="mm",
                                   name=f"ps{pgi}_{q}")
                    for i, v in enumerate(partners):
                        sl = ps[:, i, :]
                        first = True
                        if jeng == "pe":
                            nc.tensor.matmul(
                                sl, ones[0:1, :], sqr[0:1, v, :],
                                start=True, stop=False,
                            )
                            first = False
                        for ko2 in range(2):
                            nc.tensor.matmul(
                                sl,
                                x8[:, 2 * ko2:2 * ko2 + 2,
                                   coff + q * P:coff + (q + 1) * P],
                                x8[:, 2 * ko2:2 * ko2 + 2,
                                   partners[i] * SW:(partners[i] + 1) * SW],
                                start=(first and ko2 == 0), stop=(ko2 == 1),
                                perf_mode=DR,
                            )
                    if jeng == "dve":
                        nc.vector.tensor_tensor(
                            ps[:, 0:npart, :], ps[:, 0:npart, :],
                            sqb[:, partners[0]:partners[0] + npart, :],
                            Alu.add,
                        )
                    nc.scalar.activation(
                        stage[:, 0:npart, q, :], ps[:, 0:npart, :],
                        AF.Sqrt, bias=bias[:, bcol + q:bcol + q + 1],
                        scale=-2.0,
                    )
                blk0 = sum(len(p) for _, p, _, _ in PGS[:pgi])
                for i in range(npart):
                    nc.gpsimd.dma_start(out_v[blk0 + i], stage[:, i])

    nc.compile()
    _nc_cache = nc
    return nc


def kernel(embeddings):
    global LAST_EXEC_NS, LAST_RESULTS
    emb = np.ascontiguousarray(np.asarray(embeddings, dtype=np.float32))
    assert emb.shape == (N, D)

    x8_full = emb.astype(ml_dtypes.float8_e4m3)          # [N, D] quantized
    x8f = x8_full.astype(np.float32)
    sq = (x8f.astype(np.float64) ** 2).sum(axis=1).astype(np.float32)  # [N]
    x8t = np.ascontiguousarray(x8_full.T)                # [D, N] fp8

    in_maps = []
    for c in range(NCORES):
        slots = [(2 * c + s) % NSTRIP for s in range(9)] + [c + 8, c]
        x8c = np.concatenate(
            [x8t[:, g * SW:(g + 1) * SW] for g in slots], axis=1
        )
        sqs = np.concatenate([sq[g * SW:(g + 1) * SW] for g in slots])
        sqr = (-0.5 * sqs)[None, :].astype(np.float16)
        sqb = np.broadcast_to(sqr, (P, NSLOT * SW)).copy()
        bias = np.empty((P, 12), dtype=np.float32)
        for ci, g in enumerate((2 * c, 2 * c + 1, c)):
            for q in range(KO):
                bias[:, ci * 4 + q] = sq[g * SW + q * P:g * SW + (q + 1) * P]
        in_maps.append({
            "x8": np.ascontiguousarray(x8c),
            "sqr": sqr,
            "sqb": sqb,
            "bias": bias,
        })

    nc = _build()
    from concourse.bass_utils import run_bass_kernel_spmd

    kwargs = {}
    if TRACE:
        kwargs["trace"] = True
    try:
        r = run_bass_kernel_spmd(
            nc, in_maps, core_ids=list(range(NCORES)), **kwargs
        )
    except Exception:  # noqa: BLE001
        # A previously-profiled NEFF can leave one-shot NRT state that fails
        # the next execution; the failed attempt clears it.
        r = run_bass_kernel_spmd(
            nc, in_maps, core_ids=list(range(NCORES)), **kwargs
        )
    LAST_EXEC_NS = r.exec_time_ns
    LAST_RESULTS = r

    full = np.empty((N, N), dtype=np.float32)
    for c in range(NCORES):
        arr = r.results[c]["out"].reshape(NBLK, P, KO, SW)
        bi = 0
        for coff, partners, _, _ in PGS:
            u = {0: 2 * c, SW: 2 * c + 1, 10 * SW: c}[coff] % NSTRIP
            for s in partners:
                v = (2 * c + s) % NSTRIP if s <= 8 else (c + 8)
                blk = (arr[bi].transpose(1, 0, 2).reshape(SW, SW)
                       .astype(np.float32))
                full[u * SW:(u + 1) * SW, v * SW:(v + 1) * SW] = blk
                full[v * SW:(v + 1) * SW, u * SW:(u + 1) * SW] = blk.T
                bi += 1
    np.fill_diagonal(full, 0.0)
    return full[None, :, :]


# revision 8
# speedup vs baseline: 1.1314x; 1.1314x over previous
"""Symmetric-halved Euclidean distance matrix on 8 Trainium2 NeuronCores.

v2: fp8e4m3 DoubleRow gram matmuls (2x contraction per pass), norms folded in
on-device (row term via K=1 fp16 matmuls or DVE adds, column term via the
activation bias), sqrt on the scalar engine, fp16 output, host-precomputed
norms. 136 unique strip pairs (16 column strips of 512) are distributed 17 per
core: core c owns stars centered at strips 2c (offsets 0-7) and 2c+1 (offsets
0-7), plus the distance-8 pair {c, c+8}. The host mirrors each [512, 512]
block to its transposed position.

Per-core input is a local window of 11 strips (slots 0-8 = strips
2c..2c+8 mod 16, slot 9 = strip c+8, slot 10 = strip c), so the program is
SPMD-uniform with local slot addressing.
"""
import sys

sys.path.insert(0, "/opt/trn_rl_repo")

import numpy as np
import ml_dtypes

N, D, NCORES = 8192, 512, 8
P = 128
KO = D // P          # 4 contraction blocks of 128
NSTRIP = 16
SW = N // NSTRIP     # 512 strip width
NSLOT = 11           # local strips per core
NBLK = 17            # output blocks per core

# pair-groups: (center slot offset in x8 cols, [partner slots], bias base col,
#               j-term engine 'pe' | 'dve'). DVE-epilogue groups run mid-kernel
# so their add+sqrt chains overlap PE work; PE-j groups close the schedule to
# keep the pipeline drain short.
PGS = [
    (0, [0, 1], 0, "pe"),
    (0, [2, 3], 0, "pe"),
    (SW, [1, 2], 4, "dve"),
    (SW, [3, 4], 4, "dve"),
    (0, [4, 5], 0, "dve"),
    (0, [6, 7], 0, "dve"),
    (SW, [5, 6], 4, "pe"),
    (SW, [7, 8], 4, "pe"),
    (10 * SW, [9], 8, "pe"),
]

TRACE = False
LAST_EXEC_NS = None
LAST_RESULTS = None

_nc_cache = None


def _build():
    global _nc_cache
    if _nc_cache is not None:
        return _nc_cache

    import concourse.tile as tile
    from concourse import bacc, mybir

    f32 = mybir.dt.float32
    fp8 = mybir.dt.float8e4
    fp16 = mybir.dt.float16
    AF = mybir.ActivationFunctionType
    Alu = mybir.AluOpType
    DR = mybir.MatmulPerfMode.DoubleRow

    nc = bacc.Bacc("TRN2", target_bir_lowering=False)
    x8_d = nc.declare_dram_parameter("x8", [D, NSLOT * SW], fp8, isOutput=False)
    sqr_d = nc.declare_dram_parameter("sqr", [1, NSLOT * SW], fp16, isOutput=False)
    sqb_d = nc.declare_dram_parameter("sqb", [P, NSLOT * SW], fp16, isOutput=False)
    bias_d = nc.declare_dram_parameter("bias", [P, 12], f32, isOutput=False)
    out_d = nc.declare_dram_parameter("out", [NBLK * P, KO * SW], fp16, isOutput=True)

    with tile.TileContext(nc) as tc:
        with (
            tc.tile_pool(name="res", bufs=1) as res,
            tc.tile_pool(name="stg", bufs=3) as stg,
            tc.tile_pool(name="mmps", bufs=4, space="PSUM") as mmps,
        ):
            x8 = res.tile([P, KO, NSLOT * SW], fp8, tag="x8")
            sqr = res.tile([1, NSLOT, SW], fp16, tag="sqr")
            sqb = res.tile([P, NSLOT, SW], fp16, tag="sqb")
            bias = res.tile([P, 12], f32, tag="bias")
            ones = res.tile([1, P], fp16, tag="ones")
            wsrc = res.tile([P, 2, SW], fp8, tag="wsrc")

            nc.vector.memset(ones, 1.0)
            nc.vector.memset(wsrc, 0.0)
            nc.sync.dma_start(sqr, sqr_d[:].rearrange("o (s j) -> o s j", j=SW))
            nc.sync.dma_start(bias, bias_d[:])
            x8_ap = x8_d[:]

            def load_x8(c0, c1):
                nc.sync.dma_start(
                    x8[:, :, c0:c1],
                    x8_ap[:, c0:c1].rearrange("(ko p) j -> p ko j", p=P),
                )

            # fine-grained so the first pair-groups start early; sqb goes on
            # the gpsimd queue in parallel (first DVE add is ~10us in)
            nc.gpsimd.dma_start(sqb, sqb_d[:].rearrange("p (s j) -> p s j", j=SW))
            load_x8(0, 2 * SW)                       # slots 0-1
            load_x8(2 * SW, 4 * SW)                  # slots 2-3
            load_x8(4 * SW, 6 * SW)                  # slots 4-5
            load_x8(6 * SW, 9 * SW)                  # slots 6-8
            load_x8(9 * SW, 11 * SW)                 # slots 9-10

            out_v = out_d[:].rearrange("(b p) w -> b p w", p=P)

            # warmup matmuls on scratch data: keep the PE HAM-warm (2.4 GHz)
            # through the input-DMA window; they overwrite a PSUM tile that
            # the first real accumulation group resets anyway.
            wps_tile = None
            for pgi, (coff, partners, bcol, jeng) in enumerate(PGS):
                npart = len(partners)
                stage = stg.tile([P, 2, KO, SW], fp16, tag="stage",
                                 name=f"stage{pgi}")
                for q in range(KO):
                    ps = mmps.tile([P, 2, SW], f32, tag="mm",
                                   name=f"ps{pgi}_{q}")
                    if wps_tile is None:
                        wps_tile = ps
                        for w in range(28):
                            nc.tensor.matmul(
                                wps_tile[:, 0, :], wsrc[:, :, 0:P], wsrc[:],
                                start=True, stop=True, perf_mode=DR,
                                skip_group_check=True,
                            )
                    for i, v in enumerate(partners):
                        sl = ps[:, i, :]
                        first = True
                        if jeng == "pe":
                            nc.tensor.matmul(
                                sl, ones[0:1, :], sqr[0:1, v, :],
                                start=True, stop=False,
                            )
                            first = False
                        for ko2 in range(2):
                            nc.tensor.matmul(
                                sl,
                                x8[:, 2 * ko2:2 * ko2 + 2,
                                   coff + q * P:coff + (q + 1) * P],
                                x8[:, 2 * ko2:2 * ko2 + 2,
                                   partners[i] * SW:(partners[i] + 1) * SW],
                                start=(first and ko2 == 0), stop=(ko2 == 1),
                                perf_mode=DR,
                            )
                    if jeng == "dve":
                        nc.vector.tensor_tensor(
                            ps[:, 0:npart, :], ps[:, 0:npart, :],
                            sqb[:, partners[0]:partners[0] + npart, :],
                            Alu.add,
                        )
                    nc.scalar.activation(
                        stage[:, 0:npart, q, :], ps[:, 0:npart, :],
                        AF.Sqrt, bias=bias[:, bcol + q:bcol + q + 1],
                        scale=-2.0,
                    )
                blk0 = sum(len(p) for _, p, _, _ in PGS[:pgi])
                for i in range(npart):
                    nc.gpsimd.dma_start(out_v[blk0 + i], stage[:, i])

    nc.compile()
    _nc_cache = nc
    return nc


def kernel(embeddings):
    global LAST_EXEC_NS, LAST_RESULTS
    emb = np.ascontiguousarray(np.asarray(embeddings, dtype=np.float32))
    assert emb.shape == (N, D)

    x8_full = emb.astype(ml_dtypes.float8_e4m3)          # [N, D] quantized
    x8f = x8_full.astype(np.float32)
    sq = (x8f.astype(np.float64) ** 2).sum(axis=1).astype(np.float32)  # [N]
    x8t = np.ascontiguousarray(x8_full.T)                # [D, N] fp8

    in_maps = []
    for c in range(NCORES):
        slots = [(2 * c + s) % NSTRIP for s in range(9)] + [c + 8, c]
        x8c = np.concatenate(
            [x8t[:, g * SW:(g + 1) * SW] for g in slots], axis=1
        )
        sqs = np.concatenate([sq[g * SW:(g + 1) * SW] for g in slots])
        sqr = (-0.5 * sqs)[None, :].astype(np.float16)
        sqb = np.broadcast_to(sqr, (P, NSLOT * SW)).copy()
        bias = np.empty((P, 12), dtype=np.float32)
        for ci, g in enumerate((2 * c, 2 * c + 1, c)):
            for q in range(KO):
                bias[:, ci * 4 + q] = sq[g * SW + q * P:g * SW + (q + 1) * P]
        in_maps.append({
            "x8": np.ascontiguousarray(x8c),
            "sqr": sqr,
            "sqb": sqb,
            "bias": bias,
        })

    nc = _build()
    from concourse.bass_utils import run_bass_kernel_spmd

    kwargs = {}
    if TRACE:
        kwargs["trace"] = True
    try:
        r = run_bass_kernel_spmd(
            nc, in_maps, core_ids=list(range(NCORES)), **kwargs
        )
    except Exception:  # noqa: BLE001
        # A previously-profiled NEFF can leave one-shot NRT state that fails
        # the next execution; the failed attempt clears it.
        r = run_bass_kernel_spmd(
            nc, in_maps, core_ids=list(range(NCORES)), **kwargs
        )
    LAST_EXEC_NS = r.exec_time_ns
    LAST_RESULTS = r

    full = np.empty((N, N), dtype=np.float32)
    for c in range(NCORES):
        arr = r.results[c]["out"].reshape(NBLK, P, KO, SW)
        bi = 0
        for coff, partners, _, _ in PGS:
            u = {0: 2 * c, SW: 2 * c + 1, 10 * SW: c}[coff] % NSTRIP
            for s in partners:
                v = (2 * c + s) % NSTRIP if s <= 8 else (c + 8)
                blk = (arr[bi].transpose(1, 0, 2).reshape(SW, SW)
                       .astype(np.float32))
                full[u * SW:(u + 1) * SW, v * SW:(v + 1) * SW] = blk
                full[v * SW:(v + 1) * SW, u * SW:(u + 1) * SW] = blk.T
                bi += 1
    np.fill_diagonal(full, 0.0)
    return full[None, :, :]
